# revision 1
# baseline (speedup 1.0000x reference)
"""TopK SAE (Matryoshka) Trainium2 kernel.

Problem: x [65536,128] -> z_pre = relu((x - b_dec) @ W_enc.T + b_enc) [N,4096],
TopK(64) masking -> z; x_hat = z @ W_dec.T + b_dec. Returns (x_hat, z).

Strategy: data-parallel over rows across 8 NeuronCores; W_enc/W_dec replicated.
Per 128-row tile on each core:
  - encode on PE in fp16x3 (x and W split hi/lo fp16; 3 matmuls/chunk, fp32 PSUM)
    -> ~1e-6 relative accuracy vs fp32
  - relu PSUM->SBUF on ACT
  - exact top-64 threshold on DVE: top-8 of each of 64 column-groups (max8),
    then 8 rounds of max8+match_replace over the 512 candidates; the 64th
    extracted value is the threshold t
  - masked z = (z_pre >= t) * z_pre via one fused scalar_tensor_tensor pass
  - z written dense to HBM; PE transposes masked z (fp16) and decodes
    x_hatT = W_dec @ zT in 512-column slabs; b_dec added via ACT bias
"""
import sys
if '/opt/trn_rl_repo' not in sys.path:
    sys.path.insert(0, '/opt/trn_rl_repo')
import numpy as np

N, D, M, K = 65536, 128, 4096, 64
N_CORES = 8
NT = 64                 # 128-row tiles per core
NS = NT * 128           # rows per core
NGROUP = 64
GSZ = M // NGROUP

_cache = {}


def _build(with_enc_bias: bool):
    import concourse.mybir as mybir
    import concourse.tile as tile
    from concourse import bacc
    from concourse.masks import make_identity

    F32 = mybir.dt.float32
    F16 = mybir.dt.float16
    AF = mybir.ActivationFunctionType
    ALU = mybir.AluOpType

    nc = bacc.Bacc("TRN2", target_bir_lowering=False, debug=False, num_devices=N_CORES)
    xT_hi = nc.dram_tensor("xT_hi", [128, NS], F16, kind="ExternalInput")
    xT_lo = nc.dram_tensor("xT_lo", [128, NS], F16, kind="ExternalInput")
    W16d = nc.dram_tensor("W16", [128, M], F16, kind="ExternalInput")
    W16ld = nc.dram_tensor("W16l", [128, M], F16, kind="ExternalInput")
    Wd16d = nc.dram_tensor("Wd16", [M, 128], F16, kind="ExternalInput")
    bdecd = nc.dram_tensor("bdec", [128, 1], F32, kind="ExternalInput")
    if with_enc_bias:
        bencd = nc.dram_tensor("benc16", [1, M], F16, kind="ExternalInput")
        bencld = nc.dram_tensor("benc16l", [1, M], F16, kind="ExternalInput")
    z_out = nc.dram_tensor("z_out", [NS, M], F32, kind="ExternalOutput")
    xh_out = nc.dram_tensor("xh_out", [128, NS], F32, kind="ExternalOutput")

    with tile.TileContext(nc) as tc:
        with (
            tc.tile_pool(name="const", bufs=1) as cpool,
            tc.tile_pool(name="xt", bufs=4) as xtpool,
            tc.tile_pool(name="zsb", bufs=2) as zsbpool,
            tc.tile_pool(name="sel", bufs=2) as selpool,
            tc.tile_pool(name="zm", bufs=2) as zmpool,
            tc.tile_pool(name="zt", bufs=2) as ztpool,
            tc.tile_pool(name="xh", bufs=2) as xhpool,
            tc.tile_pool(name="penc", bufs=4, space="PSUM") as pencpool,
            tc.tile_pool(name="pzt", bufs=3, space="PSUM") as pztpool,
            tc.tile_pool(name="pdec", bufs=1, space="PSUM") as pdecpool,
        ):
            w16 = cpool.tile([128, M], F16, tag="w16")
            w16l = cpool.tile([128, M], F16, tag="w16l")
            for c in range(4):
                nc.sync.dma_start(w16[:, c*1024:(c+1)*1024], W16d.ap()[:, c*1024:(c+1)*1024])
                nc.sync.dma_start(w16l[:, c*1024:(c+1)*1024], W16ld.ap()[:, c*1024:(c+1)*1024])
            wd = cpool.tile([128, M], F16, tag="wd")
            for c in range(32):
                nc.sync.dma_start(wd[:, c*128:(c+1)*128], Wd16d.ap()[c*128:(c+1)*128, :])
            bdec = cpool.tile([128, 1], F32, tag="bdec")
            nc.sync.dma_start(bdec[:], bdecd.ap()[:, :])
            identf = cpool.tile([128, 128], F32, tag="ident")
            make_identity(nc, identf)
            if with_enc_bias:
                benc = cpool.tile([1, M], F16, tag="benc")
                nc.sync.dma_start(benc[:], bencd.ap()[:, :])
                bencl = cpool.tile([1, M], F16, tag="bencl")
                nc.sync.dma_start(bencl[:], bencld.ap()[:, :])
                ones16 = cpool.tile([1, 128], F16, tag="ones")
                nc.vector.memset(ones16[:], 1.0)

            for t in range(NT):
                u = t % 4
                xt = xtpool.tile([128, 128], F16, tag="xt")
                nc.sync.dma_start(xt[:], xT_hi.ap()[:, t*128:(t+1)*128])
                xtl = xtpool.tile([128, 128], F16, tag="xtl")
                nc.sync.dma_start(xtl[:], xT_lo.ap()[:, t*128:(t+1)*128])

                zsb = zsbpool.tile([128, M], F32, tag="zsb")
                for c in range(8):
                    penc = pencpool.tile([128, 512], F32, tag="penc")
                    nc.tensor.matmul(penc[:], xt[:], w16[:, c*512:(c+1)*512], start=True, stop=False)
                    nc.tensor.matmul(penc[:], xt[:], w16l[:, c*512:(c+1)*512], start=False, stop=False)
                    nc.tensor.matmul(penc[:], xtl[:], w16[:, c*512:(c+1)*512], start=False,
                                     stop=not with_enc_bias)
                    if with_enc_bias:
                        nc.tensor.matmul(penc[:], ones16[:], benc[:, c*512:(c+1)*512], start=False, stop=False)
                        nc.tensor.matmul(penc[:], ones16[:], bencl[:, c*512:(c+1)*512], start=False, stop=True)
                    nc.scalar.activation(zsb[:, c*512:(c+1)*512], penc[:], AF.Relu)

                Mt = selpool.tile([128, NGROUP*8], F32, tag="M")
                for g in range(NGROUP):
                    nc.vector.max(Mt[:, g*8:(g+1)*8], zsb[:, g*GSZ:(g+1)*GSZ])
                E = selpool.tile([128, K], F32, tag="E")
                for r in range(8):
                    nc.vector.max(E[:, r*8:(r+1)*8], Mt[:])
                    if r < 7:
                        nc.vector.match_replace(Mt[:], E[:, r*8:(r+1)*8], Mt[:], 0.0)
                thr = E[:, K-1:K]

                zm = zmpool.tile([128, M], F32, tag="zm")
                nc.vector.scalar_tensor_tensor(zm[:], zsb[:], thr, zsb[:], op0=ALU.is_ge, op1=ALU.mult)
                nc.sync.dma_start(z_out.ap()[t*128:(t+1)*128, :], zm[:])

                if u == 0:
                    zT = ztpool.tile([128, M*4], F16, tag="zT")
                for b in range(8):
                    pzt = pztpool.tile([128, 512], F32, tag="pzt")
                    for q in range(4):
                        c = b*4 + q
                        nc.tensor.transpose(pzt[:, q*128:(q+1)*128], zm[:, c*128:(c+1)*128], identf[:])
                    dst = zT[:].rearrange("p (c w) -> p c w", w=512)[:, b*4:(b+1)*4, u*128:(u+1)*128]
                    src = pzt[:].rearrange("p (c w) -> p c w", w=128)
                    nc.scalar.activation(dst, src, AF.Copy)

                if u == 3:
                    slab = t // 4
                    pxh = pdecpool.tile([128, 512], F32, tag="pxh")
                    for c in range(32):
                        nc.tensor.matmul(pxh[:], wd[:, c*128:(c+1)*128], zT[:, c*512:(c+1)*512],
                                         start=(c == 0), stop=(c == 31))
                    xh = xhpool.tile([128, 512], F32, tag="xh")
                    nc.scalar.activation(xh[:], pxh[:], AF.Identity, bias=bdec[:, 0:1])
                    nc.sync.dma_start(xh_out.ap()[:, slab*512:(slab+1)*512], xh[:])

    nc.compile()
    return nc


def _get_nc(with_enc_bias: bool):
    key = ("nc", with_enc_bias)
    if key not in _cache:
        _cache[key] = _build(with_enc_bias)
    return _cache[key]


def kernel(x, W_enc, b_enc, W_dec, b_dec, _trace=False, _trace_kwargs=None):
    from concourse.bass_utils import run_bass_kernel_spmd

    x = np.asarray(x, np.float32)
    W_enc = np.asarray(W_enc, np.float32)
    b_enc = np.asarray(b_enc, np.float32)
    W_dec = np.asarray(W_dec, np.float32)
    b_dec = np.asarray(b_dec, np.float32)

    with_enc_bias = bool(np.any(b_enc) or np.any(b_dec))
    nc = _get_nc(with_enc_bias)

    WT = W_enc.T.astype(np.float32)
    W16h = WT.astype(np.float16)
    W16l = (WT - W16h.astype(np.float32)).astype(np.float16)
    Wd16 = np.ascontiguousarray(W_dec.T).astype(np.float16)
    bdec_col = np.ascontiguousarray(b_dec.reshape(128, 1))

    shared = {"W16": W16h, "W16l": W16l, "Wd16": Wd16, "bdec": bdec_col}
    if with_enc_bias:
        bt = (b_enc - W_enc @ b_dec).astype(np.float32).reshape(1, M)
        bth = bt.astype(np.float16)
        btl = (bt - bth.astype(np.float32)).astype(np.float16)
        shared["benc16"] = bth
        shared["benc16l"] = btl

    in_maps = []
    for c in range(N_CORES):
        xT = np.ascontiguousarray(x[c*NS:(c+1)*NS].T)
        xT_hi = xT.astype(np.float16)
        xT_lo = (xT - xT_hi.astype(np.float32)).astype(np.float16)
        in_maps.append({"xT_hi": xT_hi, "xT_lo": xT_lo, **shared})

    kw = {}
    if _trace:
        kw["trace"] = True
        if _trace_kwargs:
            kw.update(_trace_kwargs)
    res = run_bass_kernel_spmd(nc, in_maps, core_ids=list(range(N_CORES)), **kw)

    z = np.concatenate([res.results[c]["z_out"] for c in range(N_CORES)], axis=0)
    x_hat = np.concatenate([res.results[c]["xh_out"].T for c in range(N_CORES)], axis=0)
    if _trace:
        return (x_hat, z), res
    return (x_hat, z)
